# revision 7
# baseline (speedup 1.0000x reference)
"""Trainium2 Bass kernel for nn_Difference (ignorematch mode).

Math: result[i,j] = sum_k a_fk[i,k] * (a_fk[i,k] > 0) * (b_fk[j,k] <= 0)
where a_fk = a @ feats.T, b_fk = b @ feats.T.  This factorizes into three
matmuls with elementwise ops between them:

    P = relu(a @ feats.T)            # [Na, K]
    Q = (b @ feats.T) <= 0           # [Nb, K], exactly {0.0, 1.0}
    result = P @ Q.T                 # [Na, Nb]

No [Na, Nb, K] tensor is ever materialized.

Sharding: 2x4 grid over the output. Core (r, q) computes
result[r*512:(r+1)*512, q*256:(q+1)*256] from a-half r and b-quarter q;
feats is replicated. Inputs are pre-transposed on host so the contraction
dim D lands on SBUF partitions (the PE array reduces over partitions).

Precision: the b-side (mask) runs in fp32 — the sign of b_fk must match the
fp32 reference exactly (min |b_fk| ~ 1e-4; bf16/tf32 would flip ~20-200 mask
bits, each costing an O(20) absolute error in the output). The a-side and
final matmul run as float32r (fast fp32 path, 1 cycle/row at N>=256).
"""

import os
import sys

import numpy as np

sys.path.insert(0, "/opt/trn_rl_repo")

import concourse.bacc as bacc  # noqa: E402
import concourse.tile as tile  # noqa: E402
from concourse import mybir  # noqa: E402
from concourse.bass_utils import run_bass_kernel_spmd  # noqa: E402

# Problem shapes (hardcoded per contract).
NA, NB, D, K = 1024, 1024, 512, 256
A_SPLIT, B_SPLIT = 2, 4  # 8 cores in a 2x4 grid over the output
IA = NA // A_SPLIT  # 512 output rows per core
JB = NB // B_SPLIT  # 256 output cols per core
P = 128
DC = D // P  # 4 contraction chunks
KC = K // P  # 2 feature-bank chunks
MC = IA // P  # 4 output row chunks

F32 = mybir.dt.float32
F32R = mybir.dt.float32r

_BUILT = None
LAST_RESULTS = None


def _build():
    nc = bacc.Bacc("TRN2", target_bir_lowering=False, debug=False)

    aT = nc.dram_tensor("aT", [D, IA], F32R, kind="ExternalInput")
    bT = nc.dram_tensor("bT", [D, JB], F32, kind="ExternalInput")
    fT = nc.dram_tensor("fT", [D, K], F32, kind="ExternalInput")
    out = nc.dram_tensor("out", [IA, JB], F32, kind="ExternalOutput")

    with tile.TileContext(nc) as tc:
        with (
            tc.tile_pool(name="ins", bufs=1) as in_pool,
            tc.tile_pool(name="mid", bufs=1) as mid_pool,
            tc.tile_pool(name="outs", bufs=1) as out_pool,
            tc.tile_pool(name="ps_b", bufs=2, space="PSUM") as ps_b_pool,
            tc.tile_pool(name="ps_a", bufs=2, space="PSUM") as ps_a_pool,
            tc.tile_pool(name="ps_o", bufs=4, space="PSUM") as ps_o_pool,
        ):
            fT_sb = in_pool.tile([P, DC, K], F32, tag="ft")
            bT_sb = in_pool.tile([P, DC, JB], F32, tag="bt")
            aT_sb = in_pool.tile([P, DC, IA], F32R, tag="at")
            nc.sync.dma_start(out=fT_sb[:], in_=fT.rearrange("(dc p) k -> p dc k", p=P))
            nc.sync.dma_start(out=bT_sb[:], in_=bT.rearrange("(dc p) j -> p dc j", p=P))
            nc.sync.dma_start(out=aT_sb[:], in_=aT.rearrange("(dc p) i -> p dc i", p=P))

            # a-side lhsT: feats rounded to f32r on-chip (b-side keeps raw f32)
            fTr_sb = mid_pool.tile([P, DC, K], F32R, tag="ftr")
            nc.vector.tensor_copy(out=fTr_sb[:], in_=fT_sb[:])

            QT_sb = mid_pool.tile([P, KC, JB], F32R, tag="qt")
            PT_sb = mid_pool.tile([P, KC, IA], F32R, tag="pt")

            # b-side in fp32: QT[k, j] = 1.0 if b_fk[j, k] <= 0 else 0.0
            for kc in range(KC):
                ps = ps_b_pool.tile([P, JB], F32, tag="psb")
                for dc in range(DC):
                    nc.tensor.matmul(
                        ps[:],
                        lhsT=fT_sb[:, dc, kc * P : (kc + 1) * P],
                        rhs=bT_sb[:, dc, :],
                        start=(dc == 0),
                        stop=(dc == DC - 1),
                    )
                nc.vector.tensor_scalar(
                    QT_sb[:, kc, :], ps[:], 0.0, None, mybir.AluOpType.is_le
                )

            # a-side in float32r: PT[k, i] = relu(a_fk[i, k])
            for kc in range(KC):
                ps = ps_a_pool.tile([P, IA], F32, tag="psa")
                for dc in range(DC):
                    nc.tensor.matmul(
                        ps[:],
                        lhsT=fTr_sb[:, dc, kc * P : (kc + 1) * P],
                        rhs=aT_sb[:, dc, :],
                        start=(dc == 0),
                        stop=(dc == DC - 1),
                    )
                nc.scalar.activation(
                    PT_sb[:, kc, :], ps[:], mybir.ActivationFunctionType.Relu
                )

            # final: out[i, j] = sum_k PT[k, i] * QT[k, j]
            out_sb = out_pool.tile([P, MC, JB], F32, tag="osb")
            for mc in range(MC):
                ps = ps_o_pool.tile([P, JB], F32, tag="pso")
                for kc in range(KC):
                    nc.tensor.matmul(
                        ps[:],
                        lhsT=PT_sb[:, kc, mc * P : (mc + 1) * P],
                        rhs=QT_sb[:, kc, :],
                        start=(kc == 0),
                        stop=(kc == KC - 1),
                    )
                evict = nc.vector.tensor_copy if mc % 2 else nc.scalar.copy
                evict(out_sb[:, mc, :], ps[:])

            nc.sync.dma_start(
                out=out.rearrange("(mc p) j -> p mc j", p=P), in_=out_sb[:]
            )

    nc.finalize()
    return nc


def _round_f32r(x):
    """Round-to-nearest-even onto FP32R (fp32 with 11 mantissa bits)."""
    xi = np.ascontiguousarray(x, dtype=np.float32).view(np.uint32)
    keep = np.uint32(0xFFFFF000)
    rounded = (xi + np.uint32(0x7FF) + ((xi >> np.uint32(12)) & np.uint32(1))) & keep
    return rounded.view(np.float32)


def kernel(a, b, feats):
    global _BUILT, LAST_RESULTS
    a = np.ascontiguousarray(a, dtype=np.float32)
    b = np.ascontiguousarray(b, dtype=np.float32)
    feats = np.ascontiguousarray(feats, dtype=np.float32)

    if _BUILT is None:
        _BUILT = _build()
    nc = _BUILT

    aT = np.ascontiguousarray(_round_f32r(a).T)  # [D, NA], pre-rounded for f32r
    bT = np.ascontiguousarray(b.T)  # [D, NB]
    fT = np.ascontiguousarray(feats.T)  # [D, K]

    in_maps = []
    for r in range(A_SPLIT):
        for q in range(B_SPLIT):
            in_maps.append(
                {
                    "aT": np.ascontiguousarray(aT[:, r * IA : (r + 1) * IA]),
                    "bT": np.ascontiguousarray(bT[:, q * JB : (q + 1) * JB]),
                    "fT": fT,
                }
            )

    kwargs = {}
    if os.environ.get("KERNEL_TRACE"):
        kwargs = dict(trace=True, trace_cores=list(range(8)))
    res = run_bass_kernel_spmd(nc, in_maps, core_ids=list(range(8)), **kwargs)
    LAST_RESULTS = res

    out = np.empty((NA, NB), dtype=np.float32)
    for c, r_map in enumerate(res.results):
        r, q = divmod(c, B_SPLIT)
        out[r * IA : (r + 1) * IA, q * JB : (q + 1) * JB] = r_map["out"]
    return out


# revision 10
# speedup vs baseline: 1.0920x; 1.0920x over previous
"""Trainium2 Bass kernel for nn_Difference (ignorematch mode).

Math: result[i,j] = sum_k a_fk[i,k] * (a_fk[i,k] > 0) * (b_fk[j,k] <= 0)
where a_fk = a @ feats.T, b_fk = b @ feats.T.  This factorizes into three
matmuls with elementwise ops between them:

    P = relu(a @ feats.T)            # [Na, K]
    Q = (b @ feats.T) <= 0           # [Nb, K], exactly {0.0, 1.0}
    result = P @ Q.T                 # [Na, Nb]

No [Na, Nb, K] tensor is ever materialized.

Sharding: 2x4 grid over the output. Core (r, q) computes
result[r*512:(r+1)*512, q*256:(q+1)*256] from a-half r and b-quarter q;
feats is replicated. Inputs are pre-transposed AND pre-packed on host so
that (a) the contraction dim D lands on SBUF partitions (the PE reduces
over partitions) and (b) every DMA reads/writes one contiguous run per
partition (max DMA efficiency).

Precision: the b-side (mask) runs in fp32 — the sign of b_fk must match
the fp32 reference (min |b_fk| ~ 1e-4; reduced precision flips mask bits,
each costing an O(20-70) absolute error in the output). The a-side and
final matmul run in fp16 (1 cycle/row on the PE, half the DMA bytes;
measured absmax error ~0.4 out of |out|max ~2400).
"""

import os
import sys

import numpy as np

sys.path.insert(0, "/opt/trn_rl_repo")

import concourse.bacc as bacc  # noqa: E402
import concourse.tile as tile  # noqa: E402
from concourse import mybir  # noqa: E402
from concourse.bass_utils import run_bass_kernel_spmd  # noqa: E402

# Problem shapes (hardcoded per contract).
NA, NB, D, K = 1024, 1024, 512, 256
A_SPLIT, B_SPLIT = 2, 4  # 8 cores in a 2x4 grid over the output
IA = NA // A_SPLIT  # 512 output rows per core
JB = NB // B_SPLIT  # 256 output cols per core
P = 128
DC = D // P  # 4 contraction chunks
KC = K // P  # 2 feature-bank chunks
MC = IA // P  # 4 output row chunks
FB = K + JB  # packed feats+b row length per (partition, dc)

F32 = mybir.dt.float32
F16 = mybir.dt.float16

_BUILT = None
LAST_RESULTS = None


def _build():
    nc = bacc.Bacc("TRN2", target_bir_lowering=False, debug=False)

    # Packed inputs: one contiguous run per partition per DMA.
    # fb[p, dc, 0:K] = feats.T[dc*128+p, :], fb[p, dc, K:] = b.T[dc*128+p, jq]
    fb0 = nc.dram_tensor("fb0", [P, 2, FB], F32, kind="ExternalInput")  # dc 0,1
    fb1 = nc.dram_tensor("fb1", [P, 2, FB], F32, kind="ExternalInput")  # dc 2,3
    ah = nc.dram_tensor("ah", [P, DC, IA], F16, kind="ExternalInput")
    out = nc.dram_tensor("out", [P, MC, JB], F32, kind="ExternalOutput")

    with tile.TileContext(nc) as tc:
        with (
            tc.tile_pool(name="ins", bufs=1) as in_pool,
            tc.tile_pool(name="mid", bufs=1) as mid_pool,
            tc.tile_pool(name="outs", bufs=1) as out_pool,
            tc.tile_pool(name="ps_b", bufs=2, space="PSUM") as ps_b_pool,
            tc.tile_pool(name="ps_a", bufs=2, space="PSUM") as ps_a_pool,
            tc.tile_pool(name="ps_o", bufs=4, space="PSUM") as ps_o_pool,
        ):
            fb_sb = [
                in_pool.tile([P, 2, FB], F32, tag="fb0", name="fb_sb0"),
                in_pool.tile([P, 2, FB], F32, tag="fb1", name="fb_sb1"),
            ]
            ah_sb = in_pool.tile([P, DC, IA], F16, tag="ah")
            nc.sync.dma_start(out=fb_sb[0][:], in_=fb0[:])
            nc.sync.dma_start(out=fb_sb[1][:], in_=fb1[:])
            nc.sync.dma_start(out=ah_sb[:], in_=ah[:])

            def fT(dc):  # feats.T chunk [128d, 256k], f32
                return fb_sb[dc // 2][:, dc % 2, 0:K]

            def bT(dc):  # b.T chunk [128d, 256j], f32
                return fb_sb[dc // 2][:, dc % 2, K:FB]

            # a-side lhsT: feats cast to fp16 on-chip (b-side keeps raw f32)
            fh_sb = mid_pool.tile([P, DC, K], F16, tag="fh")
            for h in range(2):
                nc.vector.tensor_copy(
                    out=fh_sb[:, 2 * h : 2 * h + 2, :], in_=fb_sb[h][:, :, 0:K]
                )

            QT_sb = mid_pool.tile([P, KC, JB], F16, tag="qt")
            PT_sb = mid_pool.tile([P, KC, IA], F16, tag="pt")

            # b-side in fp32: QT[k, j] = 1.0 if b_fk[j, k] <= 0 else 0.0
            # dc-major order so MMs on the first fb chunk start while the
            # second chunk's DMA is still in flight.
            ps_b = [
                ps_b_pool.tile([P, JB], F32, tag="psb", name=f"ps_b{kc}")
                for kc in range(KC)
            ]
            for dc in range(DC):
                for kc in range(KC):
                    nc.tensor.matmul(
                        ps_b[kc][:],
                        lhsT=fT(dc)[:, kc * P : (kc + 1) * P],
                        rhs=bT(dc),
                        start=(dc == 0),
                        stop=(dc == DC - 1),
                    )
            for kc in range(KC):
                nc.vector.tensor_scalar(
                    QT_sb[:, kc, :], ps_b[kc][:], 0.0, None, mybir.AluOpType.is_le
                )

            # a-side in fp16: PT[k, i] = relu(a_fk[i, k])
            for kc in range(KC):
                ps = ps_a_pool.tile([P, IA], F32, tag="psa")
                for dc in range(DC):
                    nc.tensor.matmul(
                        ps[:],
                        lhsT=fh_sb[:, dc, kc * P : (kc + 1) * P],
                        rhs=ah_sb[:, dc, :],
                        start=(dc == 0),
                        stop=(dc == DC - 1),
                    )
                nc.scalar.activation(
                    PT_sb[:, kc, :], ps[:], mybir.ActivationFunctionType.Relu
                )

            # final in fp16 (Q is exactly {0,1}): out[i, j] = sum_k PT[k,i]*QT[k,j]
            out_sb = out_pool.tile([P, MC, JB], F32, tag="osb")
            for mc in range(MC):
                ps = ps_o_pool.tile([P, JB], F32, tag="pso")
                for kc in range(KC):
                    nc.tensor.matmul(
                        ps[:],
                        lhsT=PT_sb[:, kc, mc * P : (mc + 1) * P],
                        rhs=QT_sb[:, kc, :],
                        start=(kc == 0),
                        stop=(kc == KC - 1),
                    )
                evict = nc.vector.tensor_copy if mc % 2 else nc.scalar.copy
                evict(out_sb[:, mc, :], ps[:])
                if mc == 1:
                    nc.sync.dma_start(out=out[:, 0:2, :], in_=out_sb[:, 0:2, :])
            nc.sync.dma_start(out=out[:, 2:4, :], in_=out_sb[:, 2:4, :])

    nc.finalize()
    return nc


def kernel(a, b, feats):
    global _BUILT, LAST_RESULTS
    a = np.ascontiguousarray(a, dtype=np.float32)
    b = np.ascontiguousarray(b, dtype=np.float32)
    feats = np.ascontiguousarray(feats, dtype=np.float32)

    if _BUILT is None:
        _BUILT = _build()
    nc = _BUILT

    fT_full = np.ascontiguousarray(feats.T)  # [D, K]
    bT_full = np.ascontiguousarray(b.T)  # [D, NB]
    aT_h = a.T.astype(np.float16)  # [D, NA]

    # fb per (q): [P, DC, FB] with fb[:, dc, :K] = fT rows, fb[:, dc, K:] = bT rows
    fT_r = fT_full.reshape(DC, P, K)
    bT_r = bT_full.reshape(DC, P, NB)
    aT_r = aT_h.reshape(DC, P, NA)

    in_maps = []
    for r in range(A_SPLIT):
        for q in range(B_SPLIT):
            fb = np.empty((P, DC, FB), dtype=np.float32)
            fb[:, :, 0:K] = fT_r.transpose(1, 0, 2)
            fb[:, :, K:FB] = bT_r[:, :, q * JB : (q + 1) * JB].transpose(1, 0, 2)
            ah = np.ascontiguousarray(
                aT_r[:, :, r * IA : (r + 1) * IA].transpose(1, 0, 2)
            )
            in_maps.append(
                {
                    "fb0": np.ascontiguousarray(fb[:, 0:2, :]),
                    "fb1": np.ascontiguousarray(fb[:, 2:4, :]),
                    "ah": ah,
                }
            )

    kwargs = {}
    if os.environ.get("KERNEL_TRACE"):
        kwargs = dict(trace=True, trace_cores=list(range(8)))
    res = run_bass_kernel_spmd(nc, in_maps, core_ids=list(range(8)), **kwargs)
    LAST_RESULTS = res

    out = np.empty((NA, NB), dtype=np.float32)
    for c, r_map in enumerate(res.results):
        r, q = divmod(c, B_SPLIT)
        # device out: [P, MC, JB]; rows of result tile are mc*128 + p
        tile_out = r_map["out"].transpose(1, 0, 2).reshape(IA, JB)
        out[r * IA : (r + 1) * IA, q * JB : (q + 1) * JB] = tile_out
    return out


# revision 12
# speedup vs baseline: 1.0948x; 1.0026x over previous
"""Trainium2 Bass kernel for nn_Difference (ignorematch mode).

Math: result[i,j] = sum_k a_fk[i,k] * (a_fk[i,k] > 0) * (b_fk[j,k] <= 0)
where a_fk = a @ feats.T, b_fk = b @ feats.T.  This factorizes into three
matmuls with elementwise ops between them:

    P = relu(a @ feats.T)            # [Na, K]
    Q = (b @ feats.T) <= 0           # [Nb, K], exactly {0.0, 1.0}
    result = P @ Q.T                 # [Na, Nb]

No [Na, Nb, K] tensor is ever materialized.

Sharding: 2x4 grid over the output. Core (r, q) computes
result[r*512:(r+1)*512, q*256:(q+1)*256] from a-half r and b-quarter q;
feats is replicated. Inputs are pre-transposed AND pre-packed on host so
that (a) the contraction dim D lands on SBUF partitions (the PE reduces
over partitions) and (b) every DMA reads/writes one contiguous run per
partition (max DMA efficiency).

Precision: the b-side (mask) runs in fp32 — the sign of b_fk must match
the fp32 reference (min |b_fk| ~ 1e-4; reduced precision flips mask bits,
each costing an O(20-70) absolute error in the output). The a-side and
final matmul run in fp16 (1 cycle/row on the PE, half the DMA bytes;
measured absmax error ~0.4 out of |out|max ~2400).
"""

import os
import sys

import numpy as np

sys.path.insert(0, "/opt/trn_rl_repo")

import concourse.bacc as bacc  # noqa: E402
import concourse.tile as tile  # noqa: E402
from concourse import mybir  # noqa: E402
from concourse.bass_utils import run_bass_kernel_spmd  # noqa: E402

# Problem shapes (hardcoded per contract).
NA, NB, D, K = 1024, 1024, 512, 256
A_SPLIT, B_SPLIT = 2, 4  # 8 cores in a 2x4 grid over the output
IA = NA // A_SPLIT  # 512 output rows per core
JB = NB // B_SPLIT  # 256 output cols per core
P = 128
DC = D // P  # 4 contraction chunks
KC = K // P  # 2 feature-bank chunks
MC = IA // P  # 4 output row chunks
FB = K + JB  # packed feats+b row length per (partition, dc)

F32 = mybir.dt.float32
F16 = mybir.dt.float16

_BUILT = None
LAST_RESULTS = None


def _build():
    nc = bacc.Bacc("TRN2", target_bir_lowering=False, debug=False)

    # Packed inputs: one contiguous run per partition per DMA.
    # fb[p, dc, 0:K] = feats.T[dc*128+p, :], fb[p, dc, K:] = b.T[dc*128+p, jq]
    fb0 = nc.dram_tensor("fb0", [P, 2, FB], F32, kind="ExternalInput")  # dc 0,1
    fb1 = nc.dram_tensor("fb1", [P, 2, FB], F32, kind="ExternalInput")  # dc 2,3
    ah = nc.dram_tensor("ah", [P, DC, IA], F16, kind="ExternalInput")
    out = nc.dram_tensor("out", [P, MC, JB], F32, kind="ExternalOutput")

    with tile.TileContext(nc) as tc:
        with (
            tc.tile_pool(name="ins", bufs=1) as in_pool,
            tc.tile_pool(name="mid", bufs=1) as mid_pool,
            tc.tile_pool(name="outs", bufs=1) as out_pool,
            tc.tile_pool(name="ps_b", bufs=2, space="PSUM") as ps_b_pool,
            tc.tile_pool(name="ps_a", bufs=2, space="PSUM") as ps_a_pool,
            tc.tile_pool(name="ps_o", bufs=3, space="PSUM") as ps_o_pool,
            tc.tile_pool(name="ps_w", bufs=1, space="PSUM") as ps_w_pool,
        ):
            # PE warmup: the HAM clock gate keeps the PE at 1.2 GHz until it
            # has been busy ~3.4us. Run dummy matmuls on a zeroed tile while
            # the input DMAs are in flight so the real matmuls start at 2.4.
            warm_sb = in_pool.tile([P, 512], F16, tag="warm", name="warm_sb")
            nc.vector.memset(warm_sb[:], 0.0)
            ps_w = ps_w_pool.tile([P, 512], F32, tag="psw", name="ps_w")
            for _ in range(12):
                nc.tensor.matmul(
                    ps_w[:], lhsT=warm_sb[:, 0:P], rhs=warm_sb[:], start=True, stop=True
                )
            fb_sb = [
                in_pool.tile([P, 2, FB], F32, tag="fb0", name="fb_sb0"),
                in_pool.tile([P, 2, FB], F32, tag="fb1", name="fb_sb1"),
            ]
            ah_sb = in_pool.tile([P, DC, IA], F16, tag="ah")
            nc.sync.dma_start(out=fb_sb[0][:], in_=fb0[:])
            nc.sync.dma_start(out=fb_sb[1][:], in_=fb1[:])
            nc.sync.dma_start(out=ah_sb[:], in_=ah[:])

            def fT(dc):  # feats.T chunk [128d, 256k], f32
                return fb_sb[dc // 2][:, dc % 2, 0:K]

            def bT(dc):  # b.T chunk [128d, 256j], f32
                return fb_sb[dc // 2][:, dc % 2, K:FB]

            # a-side lhsT: feats cast to fp16 on-chip (b-side keeps raw f32)
            fh_sb = mid_pool.tile([P, DC, K], F16, tag="fh")
            for h in range(2):
                nc.vector.tensor_copy(
                    out=fh_sb[:, 2 * h : 2 * h + 2, :], in_=fb_sb[h][:, :, 0:K]
                )

            QT_sb = mid_pool.tile([P, KC, JB], F16, tag="qt")
            PT_sb = mid_pool.tile([P, KC, IA], F16, tag="pt")

            # b-side in fp32: QT[k, j] = 1.0 if b_fk[j, k] <= 0 else 0.0
            # dc-major order so MMs on the first fb chunk start while the
            # second chunk's DMA is still in flight.
            ps_b = [
                ps_b_pool.tile([P, JB], F32, tag="psb", name=f"ps_b{kc}")
                for kc in range(KC)
            ]
            for dc in range(DC):
                for kc in range(KC):
                    nc.tensor.matmul(
                        ps_b[kc][:],
                        lhsT=fT(dc)[:, kc * P : (kc + 1) * P],
                        rhs=bT(dc),
                        start=(dc == 0),
                        stop=(dc == DC - 1),
                    )
            for kc in range(KC):
                nc.vector.tensor_scalar(
                    QT_sb[:, kc, :], ps_b[kc][:], 0.0, None, mybir.AluOpType.is_le
                )

            # a-side in fp16: PT[k, i] = relu(a_fk[i, k])
            for kc in range(KC):
                ps = ps_a_pool.tile([P, IA], F32, tag="psa")
                for dc in range(DC):
                    nc.tensor.matmul(
                        ps[:],
                        lhsT=fh_sb[:, dc, kc * P : (kc + 1) * P],
                        rhs=ah_sb[:, dc, :],
                        start=(dc == 0),
                        stop=(dc == DC - 1),
                    )
                nc.scalar.activation(
                    PT_sb[:, kc, :], ps[:], mybir.ActivationFunctionType.Relu
                )

            # final in fp16 (Q is exactly {0,1}): out[i, j] = sum_k PT[k,i]*QT[k,j]
            out_sb = out_pool.tile([P, MC, JB], F32, tag="osb")
            for mc in range(MC):
                ps = ps_o_pool.tile([P, JB], F32, tag="pso")
                for kc in range(KC):
                    nc.tensor.matmul(
                        ps[:],
                        lhsT=PT_sb[:, kc, mc * P : (mc + 1) * P],
                        rhs=QT_sb[:, kc, :],
                        start=(kc == 0),
                        stop=(kc == KC - 1),
                    )
                evict = nc.vector.tensor_copy if mc % 2 else nc.scalar.copy
                evict(out_sb[:, mc, :], ps[:])
                nc.sync.dma_start(
                    out=out[:, mc : mc + 1, :], in_=out_sb[:, mc : mc + 1, :]
                )

    nc.finalize()
    return nc


def kernel(a, b, feats):
    global _BUILT, LAST_RESULTS
    a = np.ascontiguousarray(a, dtype=np.float32)
    b = np.ascontiguousarray(b, dtype=np.float32)
    feats = np.ascontiguousarray(feats, dtype=np.float32)

    if _BUILT is None:
        _BUILT = _build()
    nc = _BUILT

    fT_full = np.ascontiguousarray(feats.T)  # [D, K]
    bT_full = np.ascontiguousarray(b.T)  # [D, NB]
    aT_h = a.T.astype(np.float16)  # [D, NA]

    # fb per (q): [P, DC, FB] with fb[:, dc, :K] = fT rows, fb[:, dc, K:] = bT rows
    fT_r = fT_full.reshape(DC, P, K)
    bT_r = bT_full.reshape(DC, P, NB)
    aT_r = aT_h.reshape(DC, P, NA)

    in_maps = []
    for r in range(A_SPLIT):
        for q in range(B_SPLIT):
            fb = np.empty((P, DC, FB), dtype=np.float32)
            fb[:, :, 0:K] = fT_r.transpose(1, 0, 2)
            fb[:, :, K:FB] = bT_r[:, :, q * JB : (q + 1) * JB].transpose(1, 0, 2)
            ah = np.ascontiguousarray(
                aT_r[:, :, r * IA : (r + 1) * IA].transpose(1, 0, 2)
            )
            in_maps.append(
                {
                    "fb0": np.ascontiguousarray(fb[:, 0:2, :]),
                    "fb1": np.ascontiguousarray(fb[:, 2:4, :]),
                    "ah": ah,
                }
            )

    kwargs = {}
    if os.environ.get("KERNEL_TRACE"):
        kwargs = dict(trace=True, trace_cores=list(range(8)))
    res = run_bass_kernel_spmd(nc, in_maps, core_ids=list(range(8)), **kwargs)
    LAST_RESULTS = res

    out = np.empty((NA, NB), dtype=np.float32)
    for c, r_map in enumerate(res.results):
        r, q = divmod(c, B_SPLIT)
        # device out: [P, MC, JB]; rows of result tile are mc*128 + p
        tile_out = r_map["out"].transpose(1, 0, 2).reshape(IA, JB)
        out[r * IA : (r + 1) * IA, q * JB : (q + 1) * JB] = tile_out
    return out
